# revision 22
# baseline (speedup 1.0000x reference)
"""Distributed single-head transformer block on 8 TRN2 NeuronCores.

Collective-free restructuring. Algebraic folds done on the host
(weights only):
  - FFN has no activation between its two Linears, so it collapses to a
    single matrix Wf = W2@W1; the residual h folds in as Wg = Wf + I and
    LN0's gamma folds per-column: Wg2 = Wg * g0.
  - Q/K projections collapse into B = Wq.T @ Wk, so scores = x B x.T.
    Each core holds the FULL x (replicated at input-distribution time),
    so there is no K AllGather.
  - attn @ v = (P @ x) @ Wv.T + bv (softmax rows sum to 1), so there is
    no V AllGather either: P @ x uses the same resident full x.
  - LN0 folds via LN scale invariance: LN1(acc) == LN1(acc/rstd0), so
    the LN0 correction becomes acc2 = y + mu0*s2n (+ std0*cb with
    nonzero biases) -- 1-row bf16 matmuls accumulated INTO the y PSUM.
  - LN1's MEAN also folds into the y PSUM: mu1 = (wfold @ res)/D with
    wfold = Wg2.T(1/g1) + (sum(s2n/g1)/D)*ones is just another weight
    row, accumulated during the Wv phase. Subtracting g1 (x) mu1 as a
    fold matmul leaves acc CENTERED, so the LN1 chain is a single
    Rsqrt straight off the variance PSUM and the writeback is one
    multiply per tile (+ b1n scalar-add when present).

All large matmuls run in fp8 DoubleRow (2 contraction k-tiles per
instruction, 157 TF/s). The dual-fp8 ldweights ISA check requires each
(2,128) weight pair-block to be contiguous in SBUF, so the host
pre-permutes every stationary operand into [..., 2, 128]-blocked layout;
moving operands are written [..., 2, TOK]-blocked on chip.

Scheduling (v6):
  - sync HWDGE ring (in-order): xT8 -> B8d -> xTg8 in 8 chunks; scalar
    ring carries no early DMAs; background tensors ride gpsimd SWDGE
    emitted after the xB phase (gated by a copy of xB8) so they cannot
    steal front bandwidth.
  - y + LN1 + writeback split in token halves, half-outer; half 0's
    epilogue is emitted inside half 1's m-loop so the in-order engine
    queues interleave it under half 1's matmuls.
"""

import numpy as np

P = 128
D = 1024
N = 4096
NCORES = 8
TOK = N // NCORES  # 512 tokens per core
HT = TOK // 2  # 256-token halves for the y/LN1/writeback pipeline
DK = D // P  # 8 feature tiles
KP = DK // 2  # 4 feature pair-tiles
NJ = N // P  # 32 global token tiles
JP = NJ // 2  # 16 token pair-tiles
EPS = 1e-5
WSCALE = 16.0  # fp8 range scale on B and Wv
ASCALE = 32.0  # fp8 range scale on normalized attnx
SINV = 1.0 / 512.0  # 1/(WSCALE*sqrt(D)) exp logit scale; also 1/(WSCALE*ASCALE)
WO_SCALE = 512.0  # fp8 range scale on the off-diagonal FFN fold Wo
RSCALE = 16.0  # fp8 range scale on res
YSC = WO_SCALE * RSCALE  # y PSUM accumulates at this scale

_cache = {}


def _build_nc(has_cb, has_b1n):
    import concourse.tile as tile
    from concourse import bacc, mybir
    from contextlib import ExitStack

    f32 = mybir.dt.float32
    bf16 = mybir.dt.bfloat16
    f8 = mybir.dt.float8e4
    Exp = mybir.ActivationFunctionType.Exp
    Sqrt = mybir.ActivationFunctionType.Sqrt
    Copy = mybir.ActivationFunctionType.Copy
    Square = mybir.ActivationFunctionType.Square
    DR = mybir.MatmulPerfMode.DoubleRow

    nc = bacc.Bacc("TRN2", target_bir_lowering=False, debug=False, num_devices=NCORES)

    # local shard (T-layout, pre-blocked): bf16 copy carries +bv prefolded
    # (residual only); fp8 copy is pure x for the score path
    xTb = nc.dram_tensor("xTb", [P, DK, TOK], bf16, kind="ExternalInput").ap()
    xT8 = nc.dram_tensor("xT8", [P, KP, 2, TOK], f8, kind="ExternalInput").ap()
    # full x, both layouts, fp8, host pre-permuted into pair-blocked form
    xTg8 = nc.dram_tensor("xTg8", [P, NJ, KP, 2, P], f8, kind="ExternalInput").ap()
    xg8 = nc.dram_tensor("xg8", [P, DK, JP, 2, P], f8, kind="ExternalInput").ap()
    # folded weights (pair-blocked fp8 stationaries)
    B8d = nc.dram_tensor("B8d", [P, DK, KP, 2, P], f8, kind="ExternalInput").ap()
    Wv8 = nc.dram_tensor("Wv8", [P, DK, KP, 2, P], f8, kind="ExternalInput").ap()
    Wo8 = nc.dram_tensor("Wo8", [P, DK, KP, 2, P], f8, kind="ExternalInput").ap()
    dgd = nc.dram_tensor("dgd", [P, DK, P], bf16, kind="ExternalInput").ap()
    # [wfold; invg2; b1n] blocked [P, 3, DK] (per-partition columns)
    lncon = nc.dram_tensor("lncon", [P, 3, DK], f32, kind="ExternalInput").ap()
    # 1-row-blocked bf16 stationaries for the PE outer-product folds
    w2r = nc.dram_tensor("w2r", [P, DK, 2], bf16, kind="ExternalInput").ap()
    sg2r = nc.dram_tensor("sg2r", [2, DK, P], bf16, kind="ExternalInput").ap()
    cbr = (
        nc.dram_tensor("cbr", [1, DK, P], bf16, kind="ExternalInput").ap()
        if has_cb
        else None
    )
    outT = nc.dram_tensor("outT", [2, P, DK, HT], bf16, kind="ExternalOutput").ap()

    with tile.TileContext(nc) as tc, ExitStack() as ctx:
        ctx.enter_context(
            nc.allow_low_precision("bf16 stat rows; LN-invariant rescale")
        )
        consts = ctx.enter_context(tc.tile_pool(name="consts", bufs=1))
        xin = ctx.enter_context(tc.tile_pool(name="xin", bufs=1))
        bigx = ctx.enter_context(tc.tile_pool(name="bigx", bufs=1))
        wp = ctx.enter_context(tc.tile_pool(name="wp", bufs=1))
        mid = ctx.enter_context(tc.tile_pool(name="mid", bufs=1))
        ev = ctx.enter_context(tc.tile_pool(name="ev", bufs=2))
        ps = ctx.enter_context(tc.tile_pool(name="ps", bufs=3, space="PSUM"))
        pss = ctx.enter_context(tc.tile_pool(name="pss", bufs=3, space="PSUM"))
        psb = ctx.enter_context(tc.tile_pool(name="psb", bufs=2, space="PSUM"))

        # ---- front-critical input DMAs, all on the sync HWDGE ring
        # (in-order): xT8 -> B8d (gates xB) -> xTg8 chunks (gate scores).
        xT8_sb = xin.tile([P, KP, 2, TOK], f8, tag="x8s")
        nc.sync.dma_start(out=xT8_sb, in_=xT8)
        B8_sb = wp.tile([P, DK, KP, 2, P], f8)
        for c in range(4):
            ring = nc.scalar if c % 2 == 0 else nc.sync
            ring.dma_start(
                out=B8_sb[:, 2 * c : 2 * c + 2], in_=B8d[:, 2 * c : 2 * c + 2]
            )
        xTg_sb = bigx.tile([P, NJ, KP, 2, P], f8)
        for c in range(8):
            nc.sync.dma_start(
                out=xTg_sb[:, 4 * c : 4 * c + 4], in_=xTg8[:, 4 * c : 4 * c + 4]
            )

        # ---- constants -------------------------------------------------
        ones8 = consts.tile([P, 2, 16], f8)
        nc.gpsimd.memset(ones8, 1.0)
        ones_b = consts.tile([P, 1], bf16)
        nc.vector.memset(ones_b, 1.0)
        onesr = consts.tile([1, P], bf16)
        nc.vector.memset(onesr, 1.0)
        eps_sb = consts.tile([1, 1], f32)
        nc.vector.memset(eps_sb, EPS)

        from concourse.bass import (
            AP,
            MemorySpace,
            assert_is_scalar,
            assert_partition_dims_match,
        )

        def act_raw(out, in_, func, bias=0.0, scale=1.0):
            eng = nc.scalar
            inputs = [eng.lower_ap(in_)]
            for arg in (bias, scale, 0.0):
                if isinstance(arg, AP):
                    assert_partition_dims_match(arg, in_)
                    assert_is_scalar(arg)
                    assert arg.space == MemorySpace.SBUF
                    inputs.append(eng.lower_ap(arg))
                else:
                    inputs.append(
                        mybir.ImmediateValue(dtype=mybir.dt.float32, value=arg)
                    )
            return eng.add_instruction(
                mybir.InstActivation(
                    name=eng.bass.get_next_instruction_name(),
                    func=func,
                    ins=inputs,
                    outs=[eng.lower_ap(out)],
                )
            )

        Rsqrt = mybir.ActivationFunctionType.Rsqrt
        Recip = mybir.ActivationFunctionType.Reciprocal

        _bc_n = [0]

        def bcast(row_b, tag, width=TOK):
            """[1, w] bf16 -> [P, w] bf16 broadcast via PE outer product."""
            _bc_n[0] += 1
            pt = psb.tile([P, width], f32, tag="bc", name=f"bc_{_bc_n[0]}")
            nc.tensor.matmul(pt, onesr, row_b, start=True, stop=True)
            sb = consts.tile(
                [P, width], bf16, name=f"bcs_{_bc_n[0]}", tag=f"bcs_{tag}"
            )
            nc.vector.tensor_copy(sb, pt)
            return sb

        # ---- PE warm-up: the tensor engine p-state ramps over ~3us of
        # sustained work; burn dummy DR matmuls on memset data while the
        # front DMAs are still in flight so xB runs at full clock. ------
        warm8 = consts.tile([P, 2, TOK], f8)
        nc.gpsimd.memset(warm8, 0.25)
        wpt = ps.tile([1, TOK], f32, tag="pb", name="warm")
        for i in range(20):
            nc.tensor.matmul(
                wpt,
                ones8[:, :, 0:1],
                warm8,
                start=(i == 0),
                stop=(i == 19),
                perf_mode=DR,
            )

        # ---- xB = (16B) contract x (fp8 DoubleRow) ----------------------
        xB8_sb = mid.tile([P, KP, 2, TOK], f8)
        for m in range(DK):
            pt = ps.tile([P, TOK], f32, tag="pb")
            for k in range(KP):
                nc.tensor.matmul(
                    pt,
                    B8_sb[:, m, k],
                    xT8_sb[:, k],
                    start=(k == 0),
                    stop=(k == KP - 1),
                    perf_mode=DR,
                )
            nc.scalar.activation(xB8_sb[:, m // 2, m % 2, :], pt, Copy)

        # ---- background loads on gpsimd SWDGE, gated behind a tiny copy
        # of xB8 pair 0 so their transfers kick only once the front-
        # critical sync-ring traffic is nearly done. --------------------
        gate_t = ev.tile([P, 2, 1], bf16, tag="gate")
        nc.gpsimd.tensor_copy(gate_t, xB8_sb[:, KP - 1, :, 0:1])
        lncon_sb = consts.tile([P, 3, DK], f32)
        nc.gpsimd.dma_start(out=lncon_sb, in_=lncon)
        w2r_sb = consts.tile([P, DK, 2], bf16)
        nc.gpsimd.dma_start(out=w2r_sb, in_=w2r)
        sg2r_sb = consts.tile([2, DK, P], bf16)
        nc.gpsimd.dma_start(out=sg2r_sb, in_=sg2r)
        if has_cb:
            cb_sb = consts.tile([1, DK, P], bf16)
            nc.gpsimd.dma_start(out=cb_sb, in_=cbr)
        xg_sb = bigx.tile([P, DK, JP, 2, P], f8)
        for c in range(8):
            nc.gpsimd.dma_start(out=xg_sb[:, c], in_=xg8[:, c])
        Wv8_sb = wp.tile([P, DK, KP, 2, P], f8)
        nc.gpsimd.dma_start(out=Wv8_sb, in_=Wv8)
        xTb_sb = xin.tile([P, DK, TOK], bf16)
        nc.gpsimd.dma_start(out=xTb_sb, in_=xTb)
        Wo8_sb = wp.tile([P, DK, KP, 2, P], f8)
        nc.gpsimd.dma_start(out=Wo8_sb, in_=Wo8)
        dg_sb = wp.tile([P, DK, P], bf16)
        nc.gpsimd.dma_start(out=dg_sb, in_=dgd)
        # invg2 bf16 per-partition stationary; b1n f32 scalars
        invg2_sb = consts.tile([P, 1, DK], bf16)
        nc.vector.tensor_copy(invg2_sb, lncon_sb[:, 1:2])
        b1n_sb = lncon_sb[:, 2]

        # ---- scores S^T + exp -> fp8 probs, denominator interleaved ----
        pT8 = mid.tile([P, JP, 2, TOK], f8, tag="big16")
        psd = pss.tile([1, TOK], f32, tag="psm")
        for j in range(NJ):
            pt = ps.tile([P, TOK], f32, tag="pb")
            for k in range(KP):
                nc.tensor.matmul(
                    pt,
                    xTg_sb[:, j, k],
                    xB8_sb[:, k],
                    start=(k == 0),
                    stop=(k == KP - 1),
                    perf_mode=DR,
                )
            nc.scalar.activation(pT8[:, j // 2, j % 2, :], pt, Exp, bias=0.0, scale=SINV)
            if j % 2 == 1:
                nc.tensor.matmul(
                    psd,
                    ones8[:, :, 0:1],
                    pT8[:, j // 2],
                    start=(j == 1),
                    stop=(j == NJ - 1),
                    perf_mode=DR,
                )
        rden32 = consts.tile([1, TOK], bf16)
        act_raw(rden32, psd, Recip, bias=0.0, scale=1.0 / ASCALE)

        # ---- attnx = P @ x (fp8 DoubleRow), normalized to fp8. The rden
        # broadcast matmul is issued AFTER m=0's matmuls so the PE queue
        # doesn't head-of-line block on the scalar reciprocal chain. ----
        attnx8 = xin.tile([P, KP, 2, TOK], f8, tag="x8s", name="attnx8")
        rden_b = None
        for m in range(DK):
            pt = ps.tile([P, TOK], f32, tag="pb")
            for j in range(JP):
                nc.tensor.matmul(
                    pt,
                    xg_sb[:, m, j],
                    pT8[:, j],
                    start=(j == 0),
                    stop=(j == JP - 1),
                    perf_mode=DR,
                )
            if m == 0:
                rden_b = bcast(rden32, "rden")
            nc.vector.tensor_mul(attnx8[:, m // 2, m % 2, :], pt, rden_b)

        # ---- attn_out = attnx @ (16Wv).T / 512 + (x + bv) = res.
        # psm0 (ones row) and psmW (wfold row) accumulate here: they
        # feed mu0 and the LN1 mean fold. --------------------------------
        resb = xin.tile([P, DK, TOK], bf16)
        res8 = mid.tile([P, KP, 2, TOK], f8, tag="res8")
        psmw2 = pss.tile([2, TOK], f32, tag="psm")
        psq0 = pss.tile([1, TOK], f32, tag="psm") if has_cb else None
        for m in range(DK):
            pt = ps.tile([P, TOK], f32, tag="pb")
            for k in range(KP):
                nc.tensor.matmul(
                    pt,
                    Wv8_sb[:, m, k],
                    attnx8[:, k],
                    start=(k == 0),
                    stop=(k == KP - 1),
                    perf_mode=DR,
                )
            t1 = ev.tile([P, TOK], f32, tag="sq")
            nc.scalar.activation(t1, pt, Copy, bias=0.0, scale=SINV)
            nc.vector.tensor_add(resb[:, m, :], t1, xTb_sb[:, m, :])
            if m % 2 == 0:
                nc.scalar.activation(
                    res8[:, m // 2, m % 2, :], resb[:, m, :], Copy, scale=RSCALE
                )
            else:
                nc.vector.tensor_scalar_mul(
                    res8[:, m // 2, m % 2, :], resb[:, m, :], float(RSCALE)
                )
            nc.tensor.matmul(
                psmw2, w2r_sb[:, m], resb[:, m, :],
                start=(m == 0), stop=(m == DK - 1),
            )
            if has_cb:
                sq = ev.tile([P, TOK], bf16, tag="sqb")
                nc.scalar.activation(sq, resb[:, m, :], Square)
                nc.tensor.matmul(
                    psq0, ones_b, sq, start=(m == 0), stop=(m == DK - 1)
                )

        # ---- LN0 / LN1-mean scalars feeding the y-PSUM folds: one
        # two-partition act (the mean-centering sign lives in sg2r) -----
        mv01 = consts.tile([2, TOK], bf16, tag="ln_mv01")
        act_raw(mv01, psmw2, Copy, bias=0.0, scale=YSC / D)
        if has_cb:
            e20 = consts.tile([1, TOK], f32, tag="ln_e2")
            act_raw(e20, psq0, Copy, bias=0.0, scale=1.0 / D)
            mu0f = consts.tile([1, TOK], f32, tag="ln_mu0f")
            act_raw(mu0f, psmw2[0:1], Copy, bias=0.0, scale=1.0 / D)
            mu20 = consts.tile([1, TOK], f32, tag="ln_mu2")
            nc.scalar.activation(mu20, mu0f, Square)
            nc.vector.tensor_sub(e20, e20, mu20)
            std0b = consts.tile([1, TOK], bf16, tag="ln_std0")
            act_raw(std0b, e20, Sqrt, bias=eps_sb[:])
            nc.vector.tensor_scalar_mul(std0b, std0b, float(YSC))

        # ---- y = res @ Wg2.T (bf16) + folds, token halves --------------
        acc = mid.tile([P, DK, TOK], bf16, tag="big16", name="acc")
        psq1 = [None, None]
        lnrows = [None, None]  # rstd1_b per half
        outh_t = [None, None]
        hss = [slice(0, HT), slice(HT, TOK)]

        def emit_y_half(h, hooks=None):
            hs = hss[h]
            psq1[h] = pss.tile([1, HT], f32, tag="psm", name=f"psq1{h}")
            lag = 1 if h == 0 else 0  # let mu0/nmu1 land before the first
            pend = []  # fold matmuls close a PSUM group
            hooks = hooks or {}
            for m in range(DK):
                pt = ps.tile([P, HT], f32, tag="pb", name=f"y{h}_{m}")
                for k in range(KP):
                    nc.tensor.matmul(
                        pt,
                        Wo8_sb[:, m, k],
                        res8[:, k, :, hs],
                        start=(k == 0),
                        stop=False,
                        perf_mode=DR,
                    )
                nc.tensor.matmul(
                    pt, dg_sb[:, m], resb[:, m, hs], start=False, stop=False
                )
                pend.append((m, pt))
                if len(pend) > lag:
                    _close_y(h, hs, *pend.pop(0))
                if m in hooks:
                    hooks[m]()
            while pend:
                _close_y(h, hs, *pend.pop(0))

        def _close_y(h, hs, m, pt):
            nc.tensor.matmul(
                pt, sg2r_sb[:, m], mv01[:, hs], start=False, stop=not has_cb
            )
            if has_cb:
                nc.tensor.matmul(
                    pt, cb_sb[:, m], std0b[0:1, hs], start=False, stop=True
                )
            sq1 = ev.tile([P, HT], bf16, tag="sqb")
            nc.scalar.activation(sq1, pt, Square, bias=0.0, scale=1.0 / YSC)
            nc.scalar.activation(acc[:, m, hs], pt, Copy, bias=0.0, scale=1.0 / YSC)
            nc.tensor.matmul(
                psq1[h],
                invg2_sb[:, 0, m : m + 1],
                sq1,
                start=(m == 0),
                stop=(m == DK - 1),
            )

        def emit_chain(h):
            """acc is centered, so LN1 is one Rsqrt off the variance PSUM
            plus one broadcast."""
            rstd1 = consts.tile([1, HT], bf16, tag="ln_rstd", name=f"rstd1{h}")
            act_raw(rstd1, psq1[h], Rsqrt, bias=eps_sb[:], scale=1.0 / D)
            lnrows[h] = bcast(rstd1, "rstd1", width=HT)
            outh_t[h] = mid.tile(
                [P, DK, HT], bf16, tag="outh", bufs=2, name=f"outh{h}"
            )

        def emit_wb_tile(h, m):
            """out[:, m] = acc*rstd1 (+ b1n when present)."""
            hs = hss[h]
            rstd1_b = lnrows[h]
            gp = m in (2, 5)  # gpsimd owns two tiles per half
            eng = nc.gpsimd if gp else nc.vector
            if has_b1n:
                t1 = ev.tile([P, HT], bf16, tag="ot", bufs=3)
                eng.tensor_mul(t1, acc[:, m, hs], rstd1_b)
                eng.tensor_scalar_add(outh_t[h][:, m], t1, b1n_sb[:, m : m + 1])
            else:
                eng.tensor_mul(outh_t[h][:, m], acc[:, m, hs], rstd1_b)
            if m % 2 == 1:  # flush every 2 tiles so the last DMA is small
                ring = nc.sync if m % 4 == 1 else nc.scalar
                ring.dma_start(
                    out=outT[h, :, m - 1 : m + 1],
                    in_=outh_t[h][:, m - 1 : m + 1],
                )

        # half 0 plain; half 1 interleaves half 0's chain (at m=1) and
        # writeback tiles (one per close from m=3) under its matmuls.
        emit_y_half(0)
        wb_state = {"n": 0}

        def _h1_hook_chain():
            emit_chain(0)

        def _h1_hook_wb():
            emit_wb_tile(0, wb_state["n"])
            wb_state["n"] += 1

        emit_y_half(
            1,
            hooks={
                1: _h1_hook_chain,
                3: _h1_hook_wb, 4: _h1_hook_wb, 5: _h1_hook_wb,
                6: _h1_hook_wb, 7: _h1_hook_wb,
            },
        )
        while wb_state["n"] < DK:
            _h1_hook_wb()
        emit_chain(1)
        for m in range(DK):
            emit_wb_tile(1, m)

    nc.finalize()
    return nc


def _get_nc(flags):
    key = ("nc",) + flags
    if key not in _cache:
        _cache[key] = _build_nc(*flags)
    return _cache[key]


def _pair_block_m(w):
    """[D, M] -> [P, M//P, KP, 2, P] m-major pair-blocked stationary.

    w[d, m] with d = (2*k + i)*P + p, m = mt*P + c lands at
    out[p, mt, k, i, c] so each [2, P] block is contiguous and each
    output-tile's weights are one contiguous DRAM run per partition.
    """
    Dd, M = w.shape
    return np.ascontiguousarray(
        w.reshape(Dd // (2 * P), 2, P, M // P, P).transpose(2, 3, 0, 1, 4)
    )


def _tblock(w):
    """[D, M] -> [P, D//P, M]: d = k*P + p lands at [p, k, :]."""
    Dd, M = w.shape
    return np.ascontiguousarray(w.reshape(Dd // P, P, M).transpose(1, 0, 2))


def _dg_block(dg):
    """diag(Wg2) -> [P, DK, P] bf16 per-m-tile diagonal stationaries,
    scaled so the y PSUM accumulates at x YSC."""
    import ml_dtypes

    out = np.zeros((P, DK, P), dtype=np.float32)
    for m in range(DK):
        out[np.arange(P), m, np.arange(P)] = YSC * dg[m * P : m * P + P]
    return out.astype(ml_dtypes.bfloat16)


def _make_in_maps(inputs):
    import ml_dtypes

    bf = ml_dtypes.bfloat16
    f8 = ml_dtypes.float8_e4m3

    x = np.asarray(inputs["x"], dtype=np.float64)
    Wq = np.asarray(inputs["Wq"], np.float64)
    Wk = np.asarray(inputs["Wk"], np.float64)
    Wv = np.asarray(inputs["Wv"], np.float64)
    W1 = np.asarray(inputs["W1"], np.float64)
    W2 = np.asarray(inputs["W2"], np.float64)
    g0 = np.asarray(inputs["g0"], np.float64)
    b0 = np.asarray(inputs["b0"], np.float64)
    b1 = np.asarray(inputs["b1"], np.float64)
    b2 = np.asarray(inputs["b2"], np.float64)

    xf32 = x.astype(np.float32)
    x8 = xf32.astype(f8)
    xT8f = np.ascontiguousarray(xf32.T).astype(f8)

    Wf = W2 @ W1
    Wg = Wf + np.eye(D)
    g1f = np.asarray(inputs["g1"], np.float64)
    Wg2 = Wg * g0[None, :] * g1f[:, None]
    invg = 1.0 / g1f
    dg = np.diag(Wg2).copy()
    Wo = Wg2 - np.diag(dg)
    s2n = -Wg2.sum(axis=1)
    cb = (Wg @ b0 + W2 @ b1 + b2) * g1f
    # LN1 mean fold: mu1*D = wfold @ res (+ sum(cb/g1)*std0, folded into
    # the cb stationary below)
    wfold = Wg2.T @ invg + (np.dot(s2n, invg) / D)
    c2s = np.dot(cb, invg)
    cb2 = cb - (c2s / D) * g1f
    b1nf = np.asarray(inputs["b1n"], np.float64)
    lncon = np.stack(
        [
            wfold.astype(np.float32),
            (invg * invg).astype(np.float32),
            b1nf.astype(np.float32),
        ],
        axis=0,
    )  # [3, D]
    shared = {
        "B8d": _pair_block_m((WSCALE * (Wq.T @ Wk)).astype(np.float32).astype(f8)),
        "Wv8": _pair_block_m((WSCALE * Wv.T).astype(np.float32).astype(f8)),
        "Wo8": _pair_block_m(
            (WO_SCALE * Wo.T).astype(np.float32).astype(f8)
        ),
        "dgd": _dg_block(dg),
        # [P, 3, DK]: row d = m*P + p of each vector at [p, i, m]
        "lncon": np.ascontiguousarray(
            lncon.reshape(3, DK, P).transpose(2, 0, 1)
        ),
        # [P, DK, 2] stats stationary: col0 ones (psm0), col1 wfold
        "w2r": np.ascontiguousarray(
            np.stack(
                [np.ones((DK, P), np.float32),
                 wfold.astype(np.float32).reshape(DK, P)],
                axis=-1,
            ).transpose(1, 0, 2)
        ).astype(bf),
        # [2, DK, P] fold stationary rows: [s2n; -g1]
        "sg2r": np.ascontiguousarray(
            np.stack(
                [s2n.astype(np.float32).reshape(DK, P),
                 -g1f.astype(np.float32).reshape(DK, P)],
                axis=0,
            )
        ).astype(bf),
        "cbr": np.ascontiguousarray(
            cb2.astype(np.float32).reshape(1, DK, P)
        ).astype(bf),
        # scores stationary: [p, jt, k, i, m] = x[jt*P+m, (2k+i)*P+p]
        "xTg8": np.ascontiguousarray(
            xT8f.reshape(KP, 2, P, NJ, P).transpose(2, 3, 0, 1, 4)
        ),
        # attnx stationary: [p, mt, jp, i, m] = x[(2jp+i)*P+p, mt*P+m]
        "xg8": np.ascontiguousarray(
            x8.reshape(JP, 2, P, DK, P).transpose(2, 3, 0, 1, 4)
        ),
    }
    has_cb = bool(np.any(cb != 0.0))
    has_b1n = bool(np.any(b1nf != 0.0))
    bvf = np.asarray(inputs["bv"], np.float64)
    xTbv = (x + bvf[None, :]).T.astype(np.float32)
    xT = np.ascontiguousarray(xf32.T)
    in_maps = []
    for c in range(NCORES):
        m = dict(shared)
        m["xTb"] = _tblock(
            np.ascontiguousarray(xTbv[:, c * TOK : (c + 1) * TOK]).astype(bf)
        )
        # moving operand of xB: [p, k, i, t] = x[t, (2k+i)*P+p]
        xTl = np.ascontiguousarray(xT[:, c * TOK : (c + 1) * TOK]).astype(f8)
        m["xT8"] = np.ascontiguousarray(
            xTl.reshape(KP, 2, P, TOK).transpose(2, 0, 1, 3)
        )
        if not has_cb:
            del m["cbr"]
        in_maps.append(m)
    return in_maps, (has_cb, has_b1n)


def _assemble(res):
    out = np.empty((N, D), dtype=np.float32)
    for c in range(NCORES):
        # outT [2, P, DK, HT] bf16: out[h*HT+t, m*P+p] = arr[h, p, m, t]
        arr = np.asarray(res.results[c]["outT"], dtype=np.float32)
        out[c * TOK : (c + 1) * TOK, :] = arr.transpose(0, 3, 2, 1).reshape(TOK, D)
    return out


def kernel(**inputs):
    from concourse import bass_utils

    in_maps, flags = _make_in_maps(inputs)
    nc = _get_nc(flags)
    res = bass_utils.run_bass_kernel_spmd(
        nc, in_maps, core_ids=list(range(NCORES)), trace=False
    )
    return _assemble(res)


def run_traced(inputs):
    """Like kernel() but with NTFF tracing; returns (out, exec_time_ns, results)."""
    import hookshim

    hookshim.install()
    from concourse import bass_utils

    in_maps, flags = _make_in_maps(inputs)
    nc = _get_nc(flags)
    res = bass_utils.run_bass_kernel_spmd(
        nc, in_maps, core_ids=list(range(NCORES)), trace=True
    )
    return _assemble(res), res.exec_time_ns, res


# revision 23
# speedup vs baseline: 1.0059x; 1.0059x over previous
"""Distributed single-head transformer block on 8 TRN2 NeuronCores.

Collective-free restructuring. Algebraic folds done on the host
(weights only):
  - FFN has no activation between its two Linears, so it collapses to a
    single matrix Wf = W2@W1; the residual h folds in as Wg = Wf + I and
    LN0's gamma folds per-column: Wg2 = Wg * g0.
  - Q/K projections collapse into B = Wq.T @ Wk, so scores = x B x.T.
    Each core holds the FULL x (replicated at input-distribution time),
    so there is no K AllGather.
  - attn @ v = (P @ x) @ Wv.T + bv (softmax rows sum to 1), so there is
    no V AllGather either: P @ x uses the same resident full x.
  - LN0 folds via LN scale invariance: LN1(acc) == LN1(acc/rstd0), so
    the LN0 correction becomes acc2 = y + mu0*s2n (+ std0*cb with
    nonzero biases) -- 1-row bf16 matmuls accumulated INTO the y PSUM.
  - LN1's MEAN also folds into the y PSUM: mu1 = (wfold @ res)/D with
    wfold = Wg2.T(1/g1) + (sum(s2n/g1)/D)*ones is just another weight
    row, accumulated during the Wv phase. Subtracting g1 (x) mu1 as a
    fold matmul leaves acc CENTERED, so the LN1 chain is a single
    Rsqrt straight off the variance PSUM and the writeback is one
    multiply per tile (+ b1n scalar-add when present).

All large matmuls run in fp8 DoubleRow (2 contraction k-tiles per
instruction, 157 TF/s). The dual-fp8 ldweights ISA check requires each
(2,128) weight pair-block to be contiguous in SBUF, so the host
pre-permutes every stationary operand into [..., 2, 128]-blocked layout;
moving operands are written [..., 2, TOK]-blocked on chip.

Scheduling (v6):
  - sync HWDGE ring (in-order): xT8 -> B8d -> xTg8 in 8 chunks; scalar
    ring carries no early DMAs; background tensors ride gpsimd SWDGE
    emitted after the xB phase (gated by a copy of xB8) so they cannot
    steal front bandwidth.
  - y + LN1 + writeback split in token halves, half-outer; half 0's
    epilogue is emitted inside half 1's m-loop so the in-order engine
    queues interleave it under half 1's matmuls.
"""

import numpy as np

P = 128
D = 1024
N = 4096
NCORES = 8
TOK = N // NCORES  # 512 tokens per core
HT = TOK // 2  # 256-token halves for the y/LN1/writeback pipeline
DK = D // P  # 8 feature tiles
KP = DK // 2  # 4 feature pair-tiles
NJ = N // P  # 32 global token tiles
JP = NJ // 2  # 16 token pair-tiles
EPS = 1e-5
WSCALE = 16.0  # fp8 range scale on B and Wv
ASCALE = 32.0  # fp8 range scale on normalized attnx
SINV = 1.0 / 512.0  # 1/(WSCALE*sqrt(D)) exp logit scale; also 1/(WSCALE*ASCALE)
WO_SCALE = 512.0  # fp8 range scale on the off-diagonal FFN fold Wo
RSCALE = 16.0  # fp8 range scale on res
YSC = WO_SCALE * RSCALE  # y PSUM accumulates at this scale

_cache = {}


def _build_nc(has_cb, has_b1n):
    import concourse.tile as tile
    from concourse import bacc, mybir
    from contextlib import ExitStack

    f32 = mybir.dt.float32
    bf16 = mybir.dt.bfloat16
    f8 = mybir.dt.float8e4
    Exp = mybir.ActivationFunctionType.Exp
    Sqrt = mybir.ActivationFunctionType.Sqrt
    Copy = mybir.ActivationFunctionType.Copy
    Square = mybir.ActivationFunctionType.Square
    DR = mybir.MatmulPerfMode.DoubleRow

    nc = bacc.Bacc("TRN2", target_bir_lowering=False, debug=False, num_devices=NCORES)

    # local shard (T-layout, pre-blocked): bf16 copy carries +bv prefolded
    # (residual only); fp8 copy is pure x for the score path
    xTb = nc.dram_tensor("xTb", [P, DK, TOK], bf16, kind="ExternalInput").ap()
    xT8 = nc.dram_tensor("xT8", [P, KP, 2, TOK], f8, kind="ExternalInput").ap()
    # full x, both layouts, fp8, host pre-permuted into pair-blocked form
    xTg8 = nc.dram_tensor("xTg8", [P, NJ, KP, 2, P], f8, kind="ExternalInput").ap()
    xg8 = nc.dram_tensor("xg8", [P, DK, JP, 2, P], f8, kind="ExternalInput").ap()
    # folded weights (pair-blocked fp8 stationaries)
    B8d = nc.dram_tensor("B8d", [P, DK, KP, 2, P], f8, kind="ExternalInput").ap()
    Wv8 = nc.dram_tensor("Wv8", [P, DK, KP, 2, P], f8, kind="ExternalInput").ap()
    Wo8 = nc.dram_tensor("Wo8", [P, DK, KP, 2, P], f8, kind="ExternalInput").ap()
    dgd = nc.dram_tensor("dgd", [P, DK, P], bf16, kind="ExternalInput").ap()
    # [wfold; invg2; b1n] blocked [P, 3, DK] (per-partition columns)
    lncon = nc.dram_tensor("lncon", [P, 3, DK], f32, kind="ExternalInput").ap()
    # 1-row-blocked bf16 stationaries for the PE outer-product folds
    w2r = nc.dram_tensor("w2r", [P, DK, 2], bf16, kind="ExternalInput").ap()
    sg2r = nc.dram_tensor("sg2r", [2, DK, P], bf16, kind="ExternalInput").ap()
    cbr = (
        nc.dram_tensor("cbr", [1, DK, P], bf16, kind="ExternalInput").ap()
        if has_cb
        else None
    )
    outT = nc.dram_tensor("outT", [2, P, DK, HT], bf16, kind="ExternalOutput").ap()

    with tile.TileContext(nc) as tc, ExitStack() as ctx:
        ctx.enter_context(
            nc.allow_low_precision("bf16 stat rows; LN-invariant rescale")
        )
        consts = ctx.enter_context(tc.tile_pool(name="consts", bufs=1))
        xin = ctx.enter_context(tc.tile_pool(name="xin", bufs=1))
        bigx = ctx.enter_context(tc.tile_pool(name="bigx", bufs=1))
        wp = ctx.enter_context(tc.tile_pool(name="wp", bufs=1))
        mid = ctx.enter_context(tc.tile_pool(name="mid", bufs=1))
        ev = ctx.enter_context(tc.tile_pool(name="ev", bufs=2))
        ps = ctx.enter_context(tc.tile_pool(name="ps", bufs=3, space="PSUM"))
        pss = ctx.enter_context(tc.tile_pool(name="pss", bufs=3, space="PSUM"))
        psb = ctx.enter_context(tc.tile_pool(name="psb", bufs=2, space="PSUM"))

        # ---- front-critical input DMAs, all on the sync HWDGE ring
        # (in-order): xT8 -> B8d (gates xB) -> xTg8 chunks (gate scores).
        xT8_sb = xin.tile([P, KP, 2, TOK], f8, tag="x8s")
        nc.sync.dma_start(out=xT8_sb, in_=xT8)
        B8_sb = wp.tile([P, DK, KP, 2, P], f8)
        for c in range(4):
            ring = nc.scalar if c % 2 == 0 else nc.sync
            ring.dma_start(
                out=B8_sb[:, 2 * c : 2 * c + 2], in_=B8d[:, 2 * c : 2 * c + 2]
            )
        xTg_sb = bigx.tile([P, NJ, KP, 2, P], f8)
        for c in range(8):
            nc.sync.dma_start(
                out=xTg_sb[:, 4 * c : 4 * c + 4], in_=xTg8[:, 4 * c : 4 * c + 4]
            )

        # ---- constants -------------------------------------------------
        ones8 = consts.tile([P, 2, 16], f8)
        nc.vector.memset(ones8, 1.0)
        ones_b = consts.tile([P, 1], bf16)
        nc.vector.memset(ones_b, 1.0)
        onesr = consts.tile([1, P], bf16)
        nc.vector.memset(onesr, 1.0)
        eps_sb = consts.tile([1, 1], f32)
        nc.vector.memset(eps_sb, EPS)

        from concourse.bass import (
            AP,
            MemorySpace,
            assert_is_scalar,
            assert_partition_dims_match,
        )

        def act_raw(out, in_, func, bias=0.0, scale=1.0):
            eng = nc.scalar
            inputs = [eng.lower_ap(in_)]
            for arg in (bias, scale, 0.0):
                if isinstance(arg, AP):
                    assert_partition_dims_match(arg, in_)
                    assert_is_scalar(arg)
                    assert arg.space == MemorySpace.SBUF
                    inputs.append(eng.lower_ap(arg))
                else:
                    inputs.append(
                        mybir.ImmediateValue(dtype=mybir.dt.float32, value=arg)
                    )
            return eng.add_instruction(
                mybir.InstActivation(
                    name=eng.bass.get_next_instruction_name(),
                    func=func,
                    ins=inputs,
                    outs=[eng.lower_ap(out)],
                )
            )

        Rsqrt = mybir.ActivationFunctionType.Rsqrt
        Recip = mybir.ActivationFunctionType.Reciprocal

        _bc_n = [0]

        def bcast(row_b, tag, width=TOK):
            """[1, w] bf16 -> [P, w] bf16 broadcast via PE outer product."""
            _bc_n[0] += 1
            pt = psb.tile([P, width], f32, tag="bc", name=f"bc_{_bc_n[0]}")
            nc.tensor.matmul(pt, onesr, row_b, start=True, stop=True)
            sb = consts.tile(
                [P, width], bf16, name=f"bcs_{_bc_n[0]}", tag=f"bcs_{tag}"
            )
            nc.vector.tensor_copy(sb, pt)
            return sb

        # ---- PE warm-up: the tensor engine p-state ramps over ~3us of
        # sustained work; burn dummy DR matmuls on memset data while the
        # front DMAs are still in flight so xB runs at full clock. ------
        warm8 = consts.tile([P, 2, TOK], f8)
        nc.vector.memset(warm8, 0.25)
        wpt = ps.tile([1, TOK], f32, tag="pb", name="warm")
        for i in range(12):
            nc.tensor.matmul(
                wpt,
                ones8[:, :, 0:1],
                warm8,
                start=(i == 0),
                stop=(i == 11),
                perf_mode=DR,
            )

        # ---- xB = (16B) contract x (fp8 DoubleRow) ----------------------
        xB8_sb = mid.tile([P, KP, 2, TOK], f8)
        for m in range(DK):
            pt = ps.tile([P, TOK], f32, tag="pb")
            for k in range(KP):
                nc.tensor.matmul(
                    pt,
                    B8_sb[:, m, k],
                    xT8_sb[:, k],
                    start=(k == 0),
                    stop=(k == KP - 1),
                    perf_mode=DR,
                )
            nc.scalar.activation(xB8_sb[:, m // 2, m % 2, :], pt, Copy)

        # ---- background loads on gpsimd SWDGE, gated behind a tiny copy
        # of xB8 pair 0 so their transfers kick only once the front-
        # critical sync-ring traffic is nearly done. --------------------
        gate_t = ev.tile([P, 2, 1], bf16, tag="gate")
        nc.gpsimd.tensor_copy(gate_t, xB8_sb[:, KP - 1, :, 0:1])
        lncon_sb = consts.tile([P, 3, DK], f32)
        nc.gpsimd.dma_start(out=lncon_sb, in_=lncon)
        w2r_sb = consts.tile([P, DK, 2], bf16)
        nc.gpsimd.dma_start(out=w2r_sb, in_=w2r)
        sg2r_sb = consts.tile([2, DK, P], bf16)
        nc.gpsimd.dma_start(out=sg2r_sb, in_=sg2r)
        if has_cb:
            cb_sb = consts.tile([1, DK, P], bf16)
            nc.gpsimd.dma_start(out=cb_sb, in_=cbr)
        xg_sb = bigx.tile([P, DK, JP, 2, P], f8)
        for c in range(8):
            nc.gpsimd.dma_start(out=xg_sb[:, c], in_=xg8[:, c])
        Wv8_sb = wp.tile([P, DK, KP, 2, P], f8)
        nc.gpsimd.dma_start(out=Wv8_sb, in_=Wv8)
        xTb_sb = xin.tile([P, DK, TOK], bf16)
        nc.gpsimd.dma_start(out=xTb_sb, in_=xTb)
        Wo8_sb = wp.tile([P, DK, KP, 2, P], f8)
        nc.gpsimd.dma_start(out=Wo8_sb, in_=Wo8)
        dg_sb = wp.tile([P, DK, P], bf16)
        nc.gpsimd.dma_start(out=dg_sb, in_=dgd)
        # invg2 bf16 per-partition stationary; b1n f32 scalars
        invg2_sb = consts.tile([P, 1, DK], bf16)
        nc.vector.tensor_copy(invg2_sb, lncon_sb[:, 1:2])
        b1n_sb = lncon_sb[:, 2]

        # ---- scores S^T + exp -> fp8 probs, denominator interleaved ----
        pT8 = mid.tile([P, JP, 2, TOK], f8, tag="big16")
        psd = pss.tile([1, TOK], f32, tag="psm")
        for j in range(NJ):
            pt = ps.tile([P, TOK], f32, tag="pb")
            for k in range(KP):
                nc.tensor.matmul(
                    pt,
                    xTg_sb[:, j, k],
                    xB8_sb[:, k],
                    start=(k == 0),
                    stop=(k == KP - 1),
                    perf_mode=DR,
                )
            nc.scalar.activation(pT8[:, j // 2, j % 2, :], pt, Exp, bias=0.0, scale=SINV)
            if j % 2 == 1:
                nc.tensor.matmul(
                    psd,
                    ones8[:, :, 0:1],
                    pT8[:, j // 2],
                    start=(j == 1),
                    stop=(j == NJ - 1),
                    perf_mode=DR,
                )
        rden32 = consts.tile([1, TOK], bf16)
        act_raw(rden32, psd, Recip, bias=0.0, scale=1.0 / ASCALE)

        # ---- attnx = P @ x (fp8 DoubleRow), normalized to fp8. The rden
        # broadcast matmul is issued AFTER m=0's matmuls so the PE queue
        # doesn't head-of-line block on the scalar reciprocal chain. ----
        attnx8 = xin.tile([P, KP, 2, TOK], f8, tag="x8s", name="attnx8")
        rden_b = None
        for m in range(DK):
            pt = ps.tile([P, TOK], f32, tag="pb")
            for j in range(JP):
                nc.tensor.matmul(
                    pt,
                    xg_sb[:, m, j],
                    pT8[:, j],
                    start=(j == 0),
                    stop=(j == JP - 1),
                    perf_mode=DR,
                )
            if m == 0:
                rden_b = bcast(rden32, "rden")
            nc.vector.tensor_mul(attnx8[:, m // 2, m % 2, :], pt, rden_b)

        # ---- attn_out = attnx @ (16Wv).T / 512 + (x + bv) = res.
        # psm0 (ones row) and psmW (wfold row) accumulate here: they
        # feed mu0 and the LN1 mean fold. --------------------------------
        resb = xin.tile([P, DK, TOK], bf16)
        res8 = mid.tile([P, KP, 2, TOK], f8, tag="res8")
        psmw2 = pss.tile([2, TOK], f32, tag="psm")
        psq0 = pss.tile([1, TOK], f32, tag="psm") if has_cb else None
        for m in range(DK):
            pt = ps.tile([P, TOK], f32, tag="pb")
            for k in range(KP):
                nc.tensor.matmul(
                    pt,
                    Wv8_sb[:, m, k],
                    attnx8[:, k],
                    start=(k == 0),
                    stop=(k == KP - 1),
                    perf_mode=DR,
                )
            t1 = ev.tile([P, TOK], f32, tag="sq")
            nc.scalar.activation(t1, pt, Copy, bias=0.0, scale=SINV)
            nc.vector.tensor_add(resb[:, m, :], t1, xTb_sb[:, m, :])
            if m % 2 == 0:
                nc.scalar.activation(
                    res8[:, m // 2, m % 2, :], resb[:, m, :], Copy, scale=RSCALE
                )
            else:
                nc.vector.tensor_scalar_mul(
                    res8[:, m // 2, m % 2, :], resb[:, m, :], float(RSCALE)
                )
            nc.tensor.matmul(
                psmw2, w2r_sb[:, m], resb[:, m, :],
                start=(m == 0), stop=(m == DK - 1),
            )
            if has_cb:
                sq = ev.tile([P, TOK], bf16, tag="sqb")
                nc.scalar.activation(sq, resb[:, m, :], Square)
                nc.tensor.matmul(
                    psq0, ones_b, sq, start=(m == 0), stop=(m == DK - 1)
                )

        # ---- LN0 / LN1-mean scalars feeding the y-PSUM folds: one
        # two-partition act (the mean-centering sign lives in sg2r) -----
        mv01 = consts.tile([2, TOK], bf16, tag="ln_mv01")
        act_raw(mv01, psmw2, Copy, bias=0.0, scale=YSC / D)
        if has_cb:
            e20 = consts.tile([1, TOK], f32, tag="ln_e2")
            act_raw(e20, psq0, Copy, bias=0.0, scale=1.0 / D)
            mu0f = consts.tile([1, TOK], f32, tag="ln_mu0f")
            act_raw(mu0f, psmw2[0:1], Copy, bias=0.0, scale=1.0 / D)
            mu20 = consts.tile([1, TOK], f32, tag="ln_mu2")
            nc.scalar.activation(mu20, mu0f, Square)
            nc.vector.tensor_sub(e20, e20, mu20)
            std0b = consts.tile([1, TOK], bf16, tag="ln_std0")
            act_raw(std0b, e20, Sqrt, bias=eps_sb[:])
            nc.vector.tensor_scalar_mul(std0b, std0b, float(YSC))

        # ---- y = res @ Wg2.T (bf16) + folds, token halves --------------
        acc = mid.tile([P, DK, TOK], bf16, tag="big16", name="acc")
        psq1 = [None, None]
        lnrows = [None, None]  # rstd1_b per half
        outh_t = [None, None]
        hss = [slice(0, HT), slice(HT, TOK)]

        def emit_y_half(h, hooks=None):
            hs = hss[h]
            psq1[h] = pss.tile([1, HT], f32, tag="psm", name=f"psq1{h}")
            lag = 1 if h == 0 else 0  # let mu0/nmu1 land before the first
            pend = []  # fold matmuls close a PSUM group
            hooks = hooks or {}
            for m in range(DK):
                pt = ps.tile([P, HT], f32, tag="pb", name=f"y{h}_{m}")
                for k in range(KP):
                    nc.tensor.matmul(
                        pt,
                        Wo8_sb[:, m, k],
                        res8[:, k, :, hs],
                        start=(k == 0),
                        stop=False,
                        perf_mode=DR,
                    )
                nc.tensor.matmul(
                    pt, dg_sb[:, m], resb[:, m, hs], start=False, stop=False
                )
                pend.append((m, pt))
                if len(pend) > lag:
                    _close_y(h, hs, *pend.pop(0))
                if m in hooks:
                    hooks[m]()
            while pend:
                _close_y(h, hs, *pend.pop(0))

        def _close_y(h, hs, m, pt):
            nc.tensor.matmul(
                pt, sg2r_sb[:, m], mv01[:, hs], start=False, stop=not has_cb
            )
            if has_cb:
                nc.tensor.matmul(
                    pt, cb_sb[:, m], std0b[0:1, hs], start=False, stop=True
                )
            sq1 = ev.tile([P, HT], bf16, tag="sqb")
            nc.scalar.activation(sq1, pt, Square, bias=0.0, scale=1.0 / YSC)
            nc.scalar.activation(acc[:, m, hs], pt, Copy, bias=0.0, scale=1.0 / YSC)
            nc.tensor.matmul(
                psq1[h],
                invg2_sb[:, 0, m : m + 1],
                sq1,
                start=(m == 0),
                stop=(m == DK - 1),
            )

        def emit_chain(h):
            """acc is centered, so LN1 is one Rsqrt off the variance PSUM
            plus one broadcast."""
            rstd1 = consts.tile([1, HT], bf16, tag="ln_rstd", name=f"rstd1{h}")
            act_raw(rstd1, psq1[h], Rsqrt, bias=eps_sb[:], scale=1.0 / D)
            lnrows[h] = bcast(rstd1, "rstd1", width=HT)
            outh_t[h] = mid.tile(
                [P, DK, HT], bf16, tag="outh", bufs=2, name=f"outh{h}"
            )

        def emit_wb_tile(h, m):
            """out[:, m] = acc*rstd1 (+ b1n when present)."""
            hs = hss[h]
            rstd1_b = lnrows[h]
            gp = m in (2, 5)  # gpsimd owns two tiles per half
            eng = nc.gpsimd if gp else nc.vector
            if has_b1n:
                t1 = ev.tile([P, HT], bf16, tag="ot", bufs=3)
                eng.tensor_mul(t1, acc[:, m, hs], rstd1_b)
                eng.tensor_scalar_add(outh_t[h][:, m], t1, b1n_sb[:, m : m + 1])
            else:
                eng.tensor_mul(outh_t[h][:, m], acc[:, m, hs], rstd1_b)
            if m % 2 == 1:  # flush every 2 tiles so the last DMA is small
                ring = nc.sync if m % 4 == 1 else nc.scalar
                ring.dma_start(
                    out=outT[h, :, m - 1 : m + 1],
                    in_=outh_t[h][:, m - 1 : m + 1],
                )

        # half 0 plain; half 1 interleaves half 0's chain (at m=1) and
        # writeback tiles (one per close from m=3) under its matmuls.
        emit_y_half(0)
        wb_state = {"n": 0}

        def _h1_hook_chain():
            emit_chain(0)

        def _h1_hook_wb():
            emit_wb_tile(0, wb_state["n"])
            wb_state["n"] += 1

        emit_y_half(
            1,
            hooks={
                1: _h1_hook_chain,
                3: _h1_hook_wb, 4: _h1_hook_wb, 5: _h1_hook_wb,
                6: _h1_hook_wb, 7: _h1_hook_wb,
            },
        )
        while wb_state["n"] < DK:
            _h1_hook_wb()
        emit_chain(1)
        for m in range(DK):
            emit_wb_tile(1, m)

    nc.finalize()
    return nc


def _get_nc(flags):
    key = ("nc",) + flags
    if key not in _cache:
        _cache[key] = _build_nc(*flags)
    return _cache[key]


def _pair_block_m(w):
    """[D, M] -> [P, M//P, KP, 2, P] m-major pair-blocked stationary.

    w[d, m] with d = (2*k + i)*P + p, m = mt*P + c lands at
    out[p, mt, k, i, c] so each [2, P] block is contiguous and each
    output-tile's weights are one contiguous DRAM run per partition.
    """
    Dd, M = w.shape
    return np.ascontiguousarray(
        w.reshape(Dd // (2 * P), 2, P, M // P, P).transpose(2, 3, 0, 1, 4)
    )


def _tblock(w):
    """[D, M] -> [P, D//P, M]: d = k*P + p lands at [p, k, :]."""
    Dd, M = w.shape
    return np.ascontiguousarray(w.reshape(Dd // P, P, M).transpose(1, 0, 2))


def _dg_block(dg):
    """diag(Wg2) -> [P, DK, P] bf16 per-m-tile diagonal stationaries,
    scaled so the y PSUM accumulates at x YSC."""
    import ml_dtypes

    out = np.zeros((P, DK, P), dtype=np.float32)
    for m in range(DK):
        out[np.arange(P), m, np.arange(P)] = YSC * dg[m * P : m * P + P]
    return out.astype(ml_dtypes.bfloat16)


def _make_in_maps(inputs):
    import ml_dtypes

    bf = ml_dtypes.bfloat16
    f8 = ml_dtypes.float8_e4m3

    x = np.asarray(inputs["x"], dtype=np.float64)
    Wq = np.asarray(inputs["Wq"], np.float64)
    Wk = np.asarray(inputs["Wk"], np.float64)
    Wv = np.asarray(inputs["Wv"], np.float64)
    W1 = np.asarray(inputs["W1"], np.float64)
    W2 = np.asarray(inputs["W2"], np.float64)
    g0 = np.asarray(inputs["g0"], np.float64)
    b0 = np.asarray(inputs["b0"], np.float64)
    b1 = np.asarray(inputs["b1"], np.float64)
    b2 = np.asarray(inputs["b2"], np.float64)

    xf32 = x.astype(np.float32)
    x8 = xf32.astype(f8)
    xT8f = np.ascontiguousarray(xf32.T).astype(f8)

    Wf = W2 @ W1
    Wg = Wf + np.eye(D)
    g1f = np.asarray(inputs["g1"], np.float64)
    Wg2 = Wg * g0[None, :] * g1f[:, None]
    invg = 1.0 / g1f
    dg = np.diag(Wg2).copy()
    Wo = Wg2 - np.diag(dg)
    s2n = -Wg2.sum(axis=1)
    cb = (Wg @ b0 + W2 @ b1 + b2) * g1f
    # LN1 mean fold: mu1*D = wfold @ res (+ sum(cb/g1)*std0, folded into
    # the cb stationary below)
    wfold = Wg2.T @ invg + (np.dot(s2n, invg) / D)
    c2s = np.dot(cb, invg)
    cb2 = cb - (c2s / D) * g1f
    b1nf = np.asarray(inputs["b1n"], np.float64)
    lncon = np.stack(
        [
            wfold.astype(np.float32),
            (invg * invg).astype(np.float32),
            b1nf.astype(np.float32),
        ],
        axis=0,
    )  # [3, D]
    shared = {
        "B8d": _pair_block_m((WSCALE * (Wq.T @ Wk)).astype(np.float32).astype(f8)),
        "Wv8": _pair_block_m((WSCALE * Wv.T).astype(np.float32).astype(f8)),
        "Wo8": _pair_block_m(
            (WO_SCALE * Wo.T).astype(np.float32).astype(f8)
        ),
        "dgd": _dg_block(dg),
        # [P, 3, DK]: row d = m*P + p of each vector at [p, i, m]
        "lncon": np.ascontiguousarray(
            lncon.reshape(3, DK, P).transpose(2, 0, 1)
        ),
        # [P, DK, 2] stats stationary: col0 ones (psm0), col1 wfold
        "w2r": np.ascontiguousarray(
            np.stack(
                [np.ones((DK, P), np.float32),
                 wfold.astype(np.float32).reshape(DK, P)],
                axis=-1,
            ).transpose(1, 0, 2)
        ).astype(bf),
        # [2, DK, P] fold stationary rows: [s2n; -g1]
        "sg2r": np.ascontiguousarray(
            np.stack(
                [s2n.astype(np.float32).reshape(DK, P),
                 -g1f.astype(np.float32).reshape(DK, P)],
                axis=0,
            )
        ).astype(bf),
        "cbr": np.ascontiguousarray(
            cb2.astype(np.float32).reshape(1, DK, P)
        ).astype(bf),
        # scores stationary: [p, jt, k, i, m] = x[jt*P+m, (2k+i)*P+p]
        "xTg8": np.ascontiguousarray(
            xT8f.reshape(KP, 2, P, NJ, P).transpose(2, 3, 0, 1, 4)
        ),
        # attnx stationary: [p, mt, jp, i, m] = x[(2jp+i)*P+p, mt*P+m]
        "xg8": np.ascontiguousarray(
            x8.reshape(JP, 2, P, DK, P).transpose(2, 3, 0, 1, 4)
        ),
    }
    has_cb = bool(np.any(cb != 0.0))
    has_b1n = bool(np.any(b1nf != 0.0))
    bvf = np.asarray(inputs["bv"], np.float64)
    xTbv = (x + bvf[None, :]).T.astype(np.float32)
    xT = np.ascontiguousarray(xf32.T)
    in_maps = []
    for c in range(NCORES):
        m = dict(shared)
        m["xTb"] = _tblock(
            np.ascontiguousarray(xTbv[:, c * TOK : (c + 1) * TOK]).astype(bf)
        )
        # moving operand of xB: [p, k, i, t] = x[t, (2k+i)*P+p]
        xTl = np.ascontiguousarray(xT[:, c * TOK : (c + 1) * TOK]).astype(f8)
        m["xT8"] = np.ascontiguousarray(
            xTl.reshape(KP, 2, P, TOK).transpose(2, 0, 1, 3)
        )
        if not has_cb:
            del m["cbr"]
        in_maps.append(m)
    return in_maps, (has_cb, has_b1n)


def _assemble(res):
    out = np.empty((N, D), dtype=np.float32)
    for c in range(NCORES):
        # outT [2, P, DK, HT] bf16: out[h*HT+t, m*P+p] = arr[h, p, m, t]
        arr = np.asarray(res.results[c]["outT"], dtype=np.float32)
        out[c * TOK : (c + 1) * TOK, :] = arr.transpose(0, 3, 2, 1).reshape(TOK, D)
    return out


def kernel(**inputs):
    from concourse import bass_utils

    in_maps, flags = _make_in_maps(inputs)
    nc = _get_nc(flags)
    res = bass_utils.run_bass_kernel_spmd(
        nc, in_maps, core_ids=list(range(NCORES)), trace=False
    )
    return _assemble(res)


def run_traced(inputs):
    """Like kernel() but with NTFF tracing; returns (out, exec_time_ns, results)."""
    import hookshim

    hookshim.install()
    from concourse import bass_utils

    in_maps, flags = _make_in_maps(inputs)
    nc = _get_nc(flags)
    res = bass_utils.run_bass_kernel_spmd(
        nc, in_maps, core_ids=list(range(NCORES)), trace=True
    )
    return _assemble(res), res.exec_time_ns, res
